# revision 12
# baseline (speedup 1.0000x reference)
"""Multi-head attention Trainium2 kernel (8 NeuronCores, SPMD).

Problem: B=4, S=2048, D_MODEL=1024, H=16, DIM=64 (nn_MultiHeadAttn).
Sharding: core c handles (batch b = c//2, query-row chunk c%2 of 1024).
Each core computes all 16 heads for its 1024 query rows against the full
2048 keys of its batch, then its rows of the output projection.

Key structure (vs a straightforward port):
  - The K projection is algebraically eliminated: softmax over keys is
    shift-invariant per query row, so scores == qh'.k_raw with
    qh' = q (Wq^T Wk) + bq Wk (the qh.bk term is constant per row).
    Scores contract the raw streamed k directly; only Q and V project.
  - attn*V and the output projection run in fp8e4 DoubleRow perf mode:
    two 8-bit weight planes resident at once double the contraction per
    pass (measured ~2x bf16 FLOP rate).  For attn*V the two planes are
    two adjacent key chunks; exp writes land in adjacent planes of one
    [128, 2, 1024] fp8 tile so the pairing costs no relayout.  For the
    output projection the planes are head pairs of hidden.
    The dual-fp8 LDWEIGHTS ISA requires the stationary plane stride to
    be a multiple of 128, hence vha rows are 256 wide (head A at cols
    0:64 + ones col 64, head B at 128:192 + ones col 192).
  - fp8 precision management: bv is kept OUT of the attention path (the
    weighted mean of (vh+bv) is hidden+bv exactly, so bv goes to the
    host-folded output bias bo' = bo + Wo bv_full); hidden is scaled by
    32 into fp8's normal range via a 1/32 ones column (the reciprocal of
    the sum row absorbs it), and the 1/32 rides the output ACT's scale.
  - Softmax: no max subtraction (|scores|/8 <= ~2.5, shift-invariant),
    1/8 folded into exp; sum of exps rides the ones columns of vha into
    psum row 64; exp split between ACT (hw spline) and DVE (custom
    8-stage deg-3 poly ^4), both writing fp8 directly.
"""

import sys

if "/opt/trn_rl_repo" not in sys.path:
    sys.path.insert(0, "/opt/trn_rl_repo")

import numpy as np
from contextlib import ExitStack

N_CORES = 8
B, S, D = 4, 2048, 1024
H, DIM = 16, 64
SQ = 1024          # query rows per core
NPAIR = 8          # head pairs
NKC = S // 128     # key chunks of 128
HS = 32.0          # hidden fp8 scale (ones col = 1/HS; undone in out ACT)

# deg-3 minimax fit of exp(x/32) on |x|<=20; kernel computes p(x)^4=exp(x/8).
EXPC3 = 4.98779571e-06
EXPC2 = 5.03750782e-04
EXPC1 = 3.13034249e-02
EXPC0 = 9.99313241e-01

_cache = {}


def _register_exp_op():
    """Register the custom DVE exp op (deg-3 Horner + 2 squarings, 8 ALU
    stages) in concourse's custom-DVE registry; the per-NEFF uop table is
    generated from dve_ops.OPS at compile time."""
    if "exp_op" in _cache:
        return _cache["exp_op"]
    from concourse import dve_ops
    from concourse.dve_spec import (
        Spec, Src0, C0, C1, C2, C3, sq, lower, _spill_c3_to_src1,
    )
    from concourse.dve_uop import DveOpSpec

    name = "EXP_POLY4_ANT"
    for op in dve_ops.OPS:
        if op.name == name:
            _cache["exp_op"] = op
            return op

    def _ref(in0, in1, s0, s1, imm2):
        p = ((s0 * in0 + s1) * in0 + imm2) * in0 + in1
        return (p * p) * (p * p)

    body = sq(sq(((C0 * Src0 + C1) * Src0 + C2) * Src0 + C3))
    spec = Spec(body=_spill_c3_to_src1(body), reference=_ref)
    dve_ops._SUB_OPCODE_FOR_NAME[name] = dve_ops._CUSTOM_DVE_ROW_BASE + len(dve_ops.OPS)
    shas = {}
    for ver in ("v3", "v4"):
        try:
            tmp = DveOpSpec(name=name, opcode=dve_ops.get_dve_sub_opcode(name),
                            uops=lower(spec, ver=ver), rd1_en=True)
            shas[ver] = tmp.sha(ver)
        except Exception:
            pass
    op = dve_ops.DveOp(name, spec, subdim=False, uops_sha=shas)
    dve_ops.OPS.append(op)
    dve_ops.CUSTOM_DVE_SPECS[name] = spec
    _cache["exp_op"] = op
    return op


def _build_program():
    from concourse import bacc, mybir, tile

    exp_op = _register_exp_op()

    f32 = mybir.dt.float32
    bf16 = mybir.dt.bfloat16
    f8 = mybir.dt.float8e4
    DR = mybir.MatmulPerfMode.DoubleRow
    Exp = mybir.ActivationFunctionType.Exp
    Ident = mybir.ActivationFunctionType.Identity

    nc = bacc.Bacc("TRN2", target_bir_lowering=False, debug=False)

    qT = nc.dram_tensor("qT", [D, SQ], bf16, kind="ExternalInput")
    kT = nc.dram_tensor("kT", [D, S], bf16, kind="ExternalInput")
    vT = nc.dram_tensor("vT", [D, S], bf16, kind="ExternalInput")
    wq2 = nc.dram_tensor("wq2", [128, 128], bf16, kind="ExternalInput")
    bq2 = nc.dram_tensor("bq2", [128, 1], f32, kind="ExternalInput")
    wv2 = nc.dram_tensor("wv2", [128, 128], bf16, kind="ExternalInput")
    wo8 = nc.dram_tensor("wo8", [D, D], f8, kind="ExternalInput")
    bod = nc.dram_tensor("bod", [D, 1], f32, kind="ExternalInput")
    outT = nc.dram_tensor("outT", [D, SQ], bf16, kind="ExternalOutput")

    with tile.TileContext(nc) as tc:
        with ExitStack() as ctx:
            ep = ctx.enter_context
            consts = ep(tc.tile_pool(name="consts", bufs=1))
            raw = ep(tc.tile_pool(name="raw", bufs=2))
            projq = ep(tc.tile_pool(name="projq", bufs=2))
            projv = ep(tc.tile_pool(name="projv", bufs=2))
            attn_p = ep(tc.tile_pool(name="attn", bufs=4))
            norm_p = ep(tc.tile_pool(name="norm", bufs=2))
            hid_p = ep(tc.tile_pool(name="hid", bufs=1))
            outs_p = ep(tc.tile_pool(name="outs", bufs=2))
            sc_ps = ep(tc.tile_pool(name="scps", bufs=2, space="PSUM"))
            av_ps = ep(tc.tile_pool(name="avps", bufs=2, space="PSUM"))

            def mm512(out, lhsT, rhs, start=True, stop=True):
                n = out.shape[-1]
                assert rhs.shape[-1] == n
                for j in range(0, n, 512):
                    w = min(512, n - j)
                    nc.tensor.matmul(out[..., j:j + w], lhsT, rhs[..., j:j + w],
                                     start=start, stop=stop)

            def mmdr(out, lhsT, rhs, start=True, stop=True):
                # DoubleRow matmul: rhs [P, 2, n] in 512-col chunks (psum
                # bank-aligned so each bank holds one accumulation group; the
                # fp8 moving stream is 1024 elements = the same 1KB as bf16's
                # 512-element limit, verified on hw)
                n = out.shape[-1]
                assert rhs.shape[-1] == n
                for j in range(0, n, 512):
                    w = min(512, n - j)
                    nc.tensor.matmul(out[..., j:j + w], lhsT,
                                     rhs[..., j:j + w],
                                     start=start, stop=stop, perf_mode=DR)

            # ---- constants (small ones first; wo_s is only needed at the
            # output projection so its 1MB DMA is issued after the first
            # pair's raw loads) ----
            wq2_s = consts.tile([128, 128], bf16, tag="wq2")
            nc.sync.dma_start(wq2_s[:], wq2[:, :])
            bq2_s = consts.tile([128, 1], f32, tag="bq2")
            nc.sync.dma_start(bq2_s[:], bq2[:, :])
            wv2_s = consts.tile([128, 128], bf16, tag="wv2")
            nc.sync.dma_start(wv2_s[:], wv2[:, :])
            c3t = consts.tile([128, 1], f32, tag="c3t")
            nc.vector.memset(c3t[:], EXPC0)

            hidden = hid_p.tile([128, 8, SQ], f8, tag="hidden")

            # ---- PE warm-up on a memset tile (no DMA dependency): ramps
            # the HAM clock to 8/8 from the first microsecond.
            wtile = consts.tile([128, 512], bf16, tag="wtile")
            nc.vector.memset(wtile[:], 0.01)
            warm = sc_ps.tile([128, 512], f32, tag="sc")
            for _ in range(12):
                nc.tensor.matmul(warm[:], wtile[:, 0:128],
                                 wtile[:], start=True, stop=True)

            def load_raws(pair):
                rows = slice(pair * 128, (pair + 1) * 128)
                q2 = raw.tile([128, SQ], bf16, tag="q2")
                nc.sync.dma_start(q2[:], qT[rows, :])
                k2 = raw.tile([128, S], bf16, tag="k2")
                nc.sync.dma_start(k2[:], kT[rows, :])
                v2 = raw.tile([128, S], bf16, tag="v2")
                nc.sync.dma_start(v2[:], vT[rows, :])
                return q2, k2, v2

            def qproj(q2):
                # Q' projection: qh'^T[e2, q] (bias-add on ScalarE).
                # wq2 = blockdiag(Wq^T Wk), bq2 = bq Wk: scores contract
                # qh' against RAW k (no K projection exists).
                qh = projq.tile([128, SQ], bf16, tag="qh")
                ps = sc_ps.tile([128, SQ], f32, tag="sc")
                mm512(ps[:], wq2_s[:], q2[:])
                nc.scalar.activation(qh[:], ps[:], Ident, bias=bq2_s[:])
                return qh

            nxt = load_raws(0)

            wo_s = consts.tile([128, 8, D], f8, tag="wo")
            nc.sync.dma_start(wo_s[:], wo8.rearrange("(et p) o -> p et o", p=128))
            bo_s = consts.tile([128, 8], f32, tag="bo")
            nc.sync.dma_start(bo_s[:], bod.rearrange("(ot p) one -> p (ot one)", p=128))

            qh = qproj(nxt[0])
            for pair in range(NPAIR):
                q2, k2, v2 = nxt
                if pair + 1 < NPAIR:
                    nxt = load_raws(pair + 1)

                # ---- V projection (no bv: folded into bo'): vha fp8 rows of
                # 256 (A: 0:64 + ones/HS at 64; B: 128:192 + ones/HS at 192).
                # Two key chunks per psum tile and per ACT copy.
                vha = projv.tile([128, NKC, 256], f8, tag="vha")
                nc.vector.memset(vha[:, :, 64:65], 1.0 / HS)
                nc.vector.memset(vha[:, :, 192:193], 1.0 / HS)
                for sc_i in range(0, NKC, 2):
                    psv = sc_ps.tile([128, 256], f32, tag="sc")
                    for t in range(2):
                        nc.tensor.matmul(
                            psv[:, t * 128:(t + 1) * 128],
                            v2[:, (sc_i + t) * 128:(sc_i + t + 1) * 128],
                            wv2_s[:], start=True, stop=True)
                    nc.scalar.copy(
                        vha[:, sc_i:sc_i + 2, :]
                        .rearrange("p kc (h c) -> p kc h c", h=2)[..., 0:64],
                        psv[:].rearrange("p (kc h c) -> p kc h c", kc=2, h=2))

                # ---- attention over this head pair.  The j-1 AV matmuls are
                # interleaved BETWEEN score pairs so every sc-psum buffer
                # reuse is spaced past the matmul+exp round-trip latency (the
                # PE never stalls on the 2-buffer rotation), and the exps
                # they need finished an iteration ago.
                avA = av_ps.tile([65, SQ], f32, tag="av")
                avB = av_ps.tile([65, SQ], f32, tag="av")
                ats = []
                for j in range(NKC // 2):
                    atA = attn_p.tile([128, 2, SQ], f8, tag="attnA")
                    atB = attn_p.tile([128, 2, SQ], f8, tag="attnB")
                    for sub in range(2):
                        kc = 2 * j + sub
                        ks = slice(kc * 128, (kc + 1) * 128)
                        scA = sc_ps.tile([128, SQ], f32, tag="sc")
                        scB = sc_ps.tile([128, SQ], f32, tag="sc")
                        mm512(scA[:], k2[0:64, ks], qh[0:64, :])
                        mm512(scB[:], k2[64:128, ks], qh[64:128, :])
                        if j > 0:
                            pA, pB = ats[j - 1]
                            first, last = j - 1 == 0, False
                            hv = (0, 65) if sub == 0 else (128, 193)
                            at = pA if sub == 0 else pB
                            av = avA if sub == 0 else avB
                            mmdr(av[:], vha[:, 2 * (j - 1):2 * j, hv[0]:hv[1]],
                                 at[:], start=first, stop=last)
                        nc.scalar.activation(atA[:, sub, :], scA[:], Exp,
                                             scale=0.125)
                        nc.vector._custom_dve(
                            exp_op, out=atB[:, sub, :], in0=scB[:],
                            in1=c3t[:], s0=EXPC3, s1=EXPC2, imm2=EXPC1)
                    ats.append((atA, atB))
                jl = NKC // 2 - 1
                mmdr(avA[:], vha[:, 2 * jl:2 * jl + 2, 0:65], ats[jl][0][:],
                     start=False, stop=True)
                mmdr(avB[:], vha[:, 2 * jl:2 * jl + 2, 128:193], ats[jl][1][:],
                     start=False, stop=True)

                # ---- normalize: hidden[e, q] = av[e, q] * (HS/sum[q]), fp8.
                # DVE reciprocal reads the scaled sum row straight from psum
                # partition 64; GpSimd broadcasts it to partitions 0:64; head
                # B is staged and DMA'd into hidden partitions 64:128.
                if pair + 1 < NPAIR:
                    qh = qproj(nxt[0])
                for half, av in ((0, avA), (1, avB)):
                    rbs = norm_p.tile([65, SQ], f32, tag="rbs")
                    nc.scalar.copy(rbs[64:65, :], av[64:65, :])
                    sums = norm_p.tile([1, SQ], f32, tag="sums")
                    nc.scalar.dma_start(sums[:], rbs[64:65, :])
                    recip = norm_p.tile([1, SQ], f32, tag="recip")
                    nc.vector.reciprocal_approx_fast(recip[:], sums[:])
                    rb = norm_p.tile([64, SQ], f32, tag="rb")
                    nc.gpsimd.partition_broadcast(rb[:], recip[:])
                    if half == 0:
                        nc.vector.tensor_tensor(
                            hidden[0:64, pair, :],
                            av[0:64, :], rb[:], op=mybir.AluOpType.mult)
                    else:
                        stg = norm_p.tile([64, SQ], f8, tag="stg")
                        nc.vector.tensor_tensor(
                            stg[:], av[0:64, :], rb[:],
                            op=mybir.AluOpType.mult)
                        nc.scalar.dma_start(hidden[64:128, pair, :], stg[:])

            # ---- output projection (fp8 DoubleRow over head-pair planes);
            # the 1/HS hidden scale and bo' ride the output activation.
            for ot in range(8):
                pso = sc_ps.tile([128, SQ], f32, tag="sc")
                for tp in range(4):
                    mmdr(pso[:],
                         wo_s[:, 2 * tp:2 * tp + 2, ot * 128:(ot + 1) * 128],
                         hidden[:, 2 * tp:2 * tp + 2, :],
                         start=(tp == 0), stop=(tp == 3))
                o_s = outs_p.tile([128, SQ], bf16, tag="outs")
                nc.scalar.activation(o_s[:], pso[:], Ident, scale=1.0 / HS,
                                     bias=bo_s[:, ot:ot + 1])
                nc.scalar.dma_start(outT[ot * 128:(ot + 1) * 128, :], o_s[:])

    nc.compile()
    return nc


def _get_nc():
    if "nc" not in _cache:
        _cache["nc"] = _build_program()
    return _cache["nc"]


def _prep_consts(Wq, bq, Wk, bk, Wv, bv, Wo, bo):
    import ml_dtypes
    f = np.float32
    b16 = ml_dtypes.bfloat16
    f8 = ml_dtypes.float8_e4m3

    def blockdiag2(W):
        out = np.zeros((128, 128), f)
        out[:64, :64] = W
        out[64:, 64:] = W
        return out

    Wqk = Wq.T @ Wk            # [d_in, m]: qh' = q Wqk + bq Wk
    bqk = bq @ Wk
    bv_full = np.tile(bv.astype(f), H)
    bo_adj = bo.astype(f) + Wo.astype(f) @ bv_full
    return {
        "wq2": blockdiag2(Wqk).astype(b16),
        "bq2": np.tile(bqk.astype(f), 2)[:, None].copy(),
        "wv2": blockdiag2(Wv.T).astype(b16),
        "wo8": np.ascontiguousarray(Wo.T.astype(f)).astype(f8),
        "bod": bo_adj[:, None].copy(),
    }


def kernel(q, k, v, Wq, bq, Wk, bk, Wv, bv, Wo, bo, _trace=False):
    import ml_dtypes
    b16 = ml_dtypes.bfloat16
    q = np.asarray(q, np.float32)
    k = np.asarray(k, np.float32)
    v = np.asarray(v, np.float32)
    consts = _prep_consts(
        np.asarray(Wq, np.float32), np.asarray(bq, np.float32),
        np.asarray(Wk, np.float32), np.asarray(bk, np.float32),
        np.asarray(Wv, np.float32), np.asarray(bv, np.float32),
        np.asarray(Wo, np.float32), np.asarray(bo, np.float32))

    in_maps = []
    for c in range(N_CORES):
        b, chunk = c // 2, c % 2
        m = dict(consts)
        m["qT"] = np.ascontiguousarray(
            q[b, chunk * SQ:(chunk + 1) * SQ, :].T).astype(b16)
        m["kT"] = np.ascontiguousarray(k[b].T).astype(b16)
        m["vT"] = np.ascontiguousarray(v[b].T).astype(b16)
        in_maps.append(m)

    nc = _get_nc()
    from concourse.bass_utils import run_bass_kernel_spmd
    res = run_bass_kernel_spmd(nc, in_maps, core_ids=list(range(N_CORES)),
                               trace=_trace)
    if _trace:
        kernel.last_results = res

    out = np.empty((B, S, D), np.float32)
    for c in range(N_CORES):
        b, chunk = c // 2, c % 2
        out[b, chunk * SQ:(chunk + 1) * SQ, :] = \
            res.results[c]["outT"].T.astype(np.float32)
    return out


# revision 14
# speedup vs baseline: 1.0444x; 1.0444x over previous
"""Multi-head attention Trainium2 kernel (8 NeuronCores, SPMD).

Problem: B=4, S=2048, D_MODEL=1024, H=16, DIM=64 (nn_MultiHeadAttn).
Sharding: core c handles (batch b = c//2, query-row chunk c%2 of 1024).
Each core computes all 16 heads for its 1024 query rows against the full
2048 keys of its batch, then its rows of the output projection.

Key structure (vs a straightforward port):
  - The K projection is algebraically eliminated: softmax over keys is
    shift-invariant per query row, so scores == qh'.k_raw with
    qh' = q (Wq^T Wk) + bq Wk (the qh.bk term is constant per row).
    Scores contract the raw streamed k directly; only Q and V project.
  - attn*V and the output projection run in fp8e4 DoubleRow perf mode:
    two 8-bit weight planes resident at once double the contraction per
    pass (measured ~2x bf16 FLOP rate).  For attn*V the two planes are
    two adjacent key chunks; exp writes land in adjacent planes of one
    [128, 2, 1024] fp8 tile so the pairing costs no relayout.  For the
    output projection the planes are head pairs of hidden.
    The dual-fp8 LDWEIGHTS ISA requires the stationary plane stride to
    be a multiple of 128, hence vha rows are 256 wide (head A at cols
    0:64 + ones col 64, head B at 128:192 + ones col 192).
  - fp8 precision management: bv is kept OUT of the attention path (the
    weighted mean of (vh+bv) is hidden+bv exactly, so bv goes to the
    host-folded output bias bo' = bo + Wo bv_full); hidden is scaled by
    32 into fp8's normal range via a 1/32 ones column (the reciprocal of
    the sum row absorbs it), and the 1/32 rides the output ACT's scale.
  - Softmax: no max subtraction (|scores|/8 <= ~2.5, shift-invariant),
    1/8 folded into exp; sum of exps rides the ones columns of vha into
    psum row 64; exp split between ACT (hw spline) and DVE (custom
    8-stage deg-3 poly ^4), both writing fp8 directly.
"""

import sys

if "/opt/trn_rl_repo" not in sys.path:
    sys.path.insert(0, "/opt/trn_rl_repo")

import numpy as np
from contextlib import ExitStack

N_CORES = 8
B, S, D = 4, 2048, 1024
H, DIM = 16, 64
SQ = 1024          # query rows per core
NPAIR = 8          # head pairs
NKC = S // 128     # key chunks of 128
HS = 32.0          # hidden fp8 scale (ones col = 1/HS; undone in out ACT)

# deg-3 minimax fit of exp(x/32) on |x|<=20; kernel computes p(x)^4=exp(x/8).
EXPC3 = 4.98779571e-06
EXPC2 = 5.03750782e-04
EXPC1 = 3.13034249e-02
EXPC0 = 9.99313241e-01

_cache = {}


def _register_exp_op():
    """Register the custom DVE exp op (deg-3 Horner + 2 squarings, 8 ALU
    stages) in concourse's custom-DVE registry; the per-NEFF uop table is
    generated from dve_ops.OPS at compile time."""
    if "exp_op" in _cache:
        return _cache["exp_op"]
    from concourse import dve_ops
    from concourse.dve_spec import (
        Spec, Src0, C0, C1, C2, C3, sq, lower, _spill_c3_to_src1,
    )
    from concourse.dve_uop import DveOpSpec

    name = "EXP_POLY4_ANT"
    for op in dve_ops.OPS:
        if op.name == name:
            _cache["exp_op"] = op
            return op

    def _ref(in0, in1, s0, s1, imm2):
        p = ((s0 * in0 + s1) * in0 + imm2) * in0 + in1
        return (p * p) * (p * p)

    body = sq(sq(((C0 * Src0 + C1) * Src0 + C2) * Src0 + C3))
    spec = Spec(body=_spill_c3_to_src1(body), reference=_ref)
    dve_ops._SUB_OPCODE_FOR_NAME[name] = dve_ops._CUSTOM_DVE_ROW_BASE + len(dve_ops.OPS)
    shas = {}
    for ver in ("v3", "v4"):
        try:
            tmp = DveOpSpec(name=name, opcode=dve_ops.get_dve_sub_opcode(name),
                            uops=lower(spec, ver=ver), rd1_en=True)
            shas[ver] = tmp.sha(ver)
        except Exception:
            pass
    op = dve_ops.DveOp(name, spec, subdim=False, uops_sha=shas)
    dve_ops.OPS.append(op)
    dve_ops.CUSTOM_DVE_SPECS[name] = spec
    _cache["exp_op"] = op
    return op


def _build_program():
    from concourse import bacc, mybir, tile

    exp_op = _register_exp_op()

    f32 = mybir.dt.float32
    bf16 = mybir.dt.bfloat16
    f8 = mybir.dt.float8e4
    DR = mybir.MatmulPerfMode.DoubleRow
    Exp = mybir.ActivationFunctionType.Exp
    Ident = mybir.ActivationFunctionType.Identity

    nc = bacc.Bacc("TRN2", target_bir_lowering=False, debug=False)

    qT = nc.dram_tensor("qT", [D, SQ], bf16, kind="ExternalInput")
    kT = nc.dram_tensor("kT", [D, S], bf16, kind="ExternalInput")
    vT = nc.dram_tensor("vT", [D, S], bf16, kind="ExternalInput")
    wq2 = nc.dram_tensor("wq2", [128, 128], bf16, kind="ExternalInput")
    bq2 = nc.dram_tensor("bq2", [128, 1], f32, kind="ExternalInput")
    wv2 = nc.dram_tensor("wv2", [128, 128], bf16, kind="ExternalInput")
    wo8 = nc.dram_tensor("wo8", [D, D], f8, kind="ExternalInput")
    bod = nc.dram_tensor("bod", [D, 1], f32, kind="ExternalInput")
    outT = nc.dram_tensor("outT", [D, SQ], bf16, kind="ExternalOutput")

    with tile.TileContext(nc) as tc:
        with ExitStack() as ctx:
            ep = ctx.enter_context
            consts = ep(tc.tile_pool(name="consts", bufs=1))
            raw = ep(tc.tile_pool(name="raw", bufs=2))
            projq = ep(tc.tile_pool(name="projq", bufs=2))
            projv = ep(tc.tile_pool(name="projv", bufs=2))
            attn_p = ep(tc.tile_pool(name="attn", bufs=4))
            norm_p = ep(tc.tile_pool(name="norm", bufs=2))
            hid_p = ep(tc.tile_pool(name="hid", bufs=1))
            outs_p = ep(tc.tile_pool(name="outs", bufs=2))
            sc_ps = ep(tc.tile_pool(name="scps", bufs=2, space="PSUM"))
            av_ps = ep(tc.tile_pool(name="avps", bufs=2, space="PSUM"))

            def mm512(out, lhsT, rhs, start=True, stop=True):
                n = out.shape[-1]
                assert rhs.shape[-1] == n
                for j in range(0, n, 512):
                    w = min(512, n - j)
                    nc.tensor.matmul(out[..., j:j + w], lhsT, rhs[..., j:j + w],
                                     start=start, stop=stop)

            def mmdr(out, lhsT, rhs, start=True, stop=True):
                # DoubleRow matmul: rhs [P, 2, n] in 512-col chunks (psum
                # bank-aligned so each bank holds one accumulation group; the
                # fp8 moving stream is 1024 elements = the same 1KB as bf16's
                # 512-element limit, verified on hw)
                n = out.shape[-1]
                assert rhs.shape[-1] == n
                for j in range(0, n, 512):
                    w = min(512, n - j)
                    nc.tensor.matmul(out[..., j:j + w], lhsT,
                                     rhs[..., j:j + w],
                                     start=start, stop=stop, perf_mode=DR)

            # ---- constants (small ones first; wo_s is only needed at the
            # output projection so its 1MB DMA is issued after the first
            # pair's raw loads) ----
            wq2_s = consts.tile([128, 128], bf16, tag="wq2")
            nc.sync.dma_start(wq2_s[:], wq2[:, :])
            bq2_s = consts.tile([128, 1], f32, tag="bq2")
            nc.sync.dma_start(bq2_s[:], bq2[:, :])
            wv2_s = consts.tile([128, 128], bf16, tag="wv2")
            nc.sync.dma_start(wv2_s[:], wv2[:, :])
            c3t = consts.tile([128, 1], f32, tag="c3t")
            nc.vector.memset(c3t[:], EXPC0)

            hidden0 = hid_p.tile([128, 2, SQ], f8, tag="hidden0")
            hidden1 = hid_p.tile([128, 2, SQ], f8, tag="hidden1")
            hidden2 = hid_p.tile([128, 2, SQ], f8, tag="hidden2")
            hidden3 = hid_p.tile([128, 2, SQ], f8, tag="hidden3")
            hiddens = [hidden0, hidden1, hidden2, hidden3]

            # ---- PE warm-up on a memset tile (no DMA dependency): ramps
            # the HAM clock to 8/8 from the first microsecond.
            wtile = consts.tile([128, 512], bf16, tag="wtile")
            nc.vector.memset(wtile[:], 0.01)
            warm = sc_ps.tile([128, 512], f32, tag="sc")
            for _ in range(12):
                nc.tensor.matmul(warm[:], wtile[:, 0:128],
                                 wtile[:], start=True, stop=True)

            def load_raws(pair):
                rows = slice(pair * 128, (pair + 1) * 128)
                q2 = raw.tile([128, SQ], bf16, tag="q2")
                nc.sync.dma_start(q2[:], qT[rows, :])
                k2 = raw.tile([128, S], bf16, tag="k2")
                nc.sync.dma_start(k2[:], kT[rows, :])
                v2 = raw.tile([128, S], bf16, tag="v2")
                nc.sync.dma_start(v2[:], vT[rows, :])
                return q2, k2, v2

            def qproj(q2):
                # Q' projection: qh'^T[e2, q] (bias-add on ScalarE).
                # wq2 = blockdiag(Wq^T Wk), bq2 = bq Wk: scores contract
                # qh' against RAW k (no K projection exists).
                qh = projq.tile([128, SQ], bf16, tag="qh")
                ps = sc_ps.tile([128, SQ], f32, tag="sc")
                mm512(ps[:], wq2_s[:], q2[:])
                nc.scalar.activation(qh[:], ps[:], Ident, bias=bq2_s[:])
                return qh

            nxt = load_raws(0)

            wo_s = consts.tile([128, 8, D], f8, tag="wo")
            nc.sync.dma_start(wo_s[:], wo8.rearrange("(et p) o -> p et o", p=128))
            bo_s = consts.tile([128, 8], f32, tag="bo")
            nc.sync.dma_start(bo_s[:], bod.rearrange("(ot p) one -> p (ot one)", p=128))

            qh = qproj(nxt[0])
            for pair in range(NPAIR):
                q2, k2, v2 = nxt
                if pair + 1 < NPAIR:
                    nxt = load_raws(pair + 1)

                # ---- V projection (no bv: folded into bo'): vha fp8 rows of
                # 256 (A: 0:64 + ones/HS at 64; B: 128:192 + ones/HS at 192).
                # Two key chunks per psum tile and per ACT copy.
                vha = projv.tile([128, NKC, 256], f8, tag="vha")
                nc.vector.memset(vha[:, :, 64:65], 1.0 / HS)
                nc.vector.memset(vha[:, :, 192:193], 1.0 / HS)
                for sc_i in range(0, NKC, 2):
                    psv = sc_ps.tile([128, 256], f32, tag="sc")
                    for t in range(2):
                        nc.tensor.matmul(
                            psv[:, t * 128:(t + 1) * 128],
                            v2[:, (sc_i + t) * 128:(sc_i + t + 1) * 128],
                            wv2_s[:], start=True, stop=True)
                    nc.scalar.copy(
                        vha[:, sc_i:sc_i + 2, :]
                        .rearrange("p kc (h c) -> p kc h c", h=2)[..., 0:64],
                        psv[:].rearrange("p (kc h c) -> p kc h c", kc=2, h=2))

                # ---- attention over this head pair.  The j-1 AV matmuls are
                # interleaved BETWEEN score pairs so every sc-psum buffer
                # reuse is spaced past the matmul+exp round-trip latency (the
                # PE never stalls on the 2-buffer rotation), and the exps
                # they need finished an iteration ago.
                avA = av_ps.tile([65, SQ], f32, tag="av")
                avB = av_ps.tile([65, SQ], f32, tag="av")
                ats = []
                for j in range(NKC // 2):
                    atA = attn_p.tile([128, 2, SQ], f8, tag="attnA")
                    atB = attn_p.tile([128, 2, SQ], f8, tag="attnB")
                    for sub in range(2):
                        kc = 2 * j + sub
                        ks = slice(kc * 128, (kc + 1) * 128)
                        scA = sc_ps.tile([128, SQ], f32, tag="sc")
                        scB = sc_ps.tile([128, SQ], f32, tag="sc")
                        mm512(scA[:], k2[0:64, ks], qh[0:64, :])
                        mm512(scB[:], k2[64:128, ks], qh[64:128, :])
                        if j > 0:
                            pA, pB = ats[j - 1]
                            first, last = j - 1 == 0, False
                            hv = (0, 65) if sub == 0 else (128, 193)
                            at = pA if sub == 0 else pB
                            av = avA if sub == 0 else avB
                            mmdr(av[:], vha[:, 2 * (j - 1):2 * j, hv[0]:hv[1]],
                                 at[:], start=first, stop=last)
                        nc.scalar.activation(atA[:, sub, :], scA[:], Exp,
                                             scale=0.125)
                        nc.vector._custom_dve(
                            exp_op, out=atB[:, sub, :], in0=scB[:],
                            in1=c3t[:], s0=EXPC3, s1=EXPC2, imm2=EXPC1)
                    ats.append((atA, atB))
                jl = NKC // 2 - 1
                mmdr(avA[:], vha[:, 2 * jl:2 * jl + 2, 0:65], ats[jl][0][:],
                     start=False, stop=True)
                mmdr(avB[:], vha[:, 2 * jl:2 * jl + 2, 128:193], ats[jl][1][:],
                     start=False, stop=True)

                # ---- normalize: hidden[e, q] = av[e, q] * (HS/sum[q]), fp8.
                # DVE reciprocal reads the scaled sum row straight from psum
                # partition 64; GpSimd broadcasts it to partitions 0:64; head
                # B is staged and DMA'd into hidden partitions 64:128.
                if pair + 1 < NPAIR:
                    qh = qproj(nxt[0])
                hid = hiddens[pair // 2]
                hslot = pair % 2
                for half, av in ((1, avB), (0, avA)):
                    rbs = norm_p.tile([65, SQ], f32, tag="rbs")
                    nc.scalar.copy(rbs[64:65, :], av[64:65, :])
                    sums = norm_p.tile([1, SQ], f32, tag="sums")
                    nc.scalar.dma_start(sums[:], rbs[64:65, :])
                    recip = norm_p.tile([1, SQ], f32, tag="recip")
                    nc.vector.reciprocal_approx_fast(recip[:], sums[:])
                    rb = norm_p.tile([64, SQ], f32, tag="rb")
                    nc.gpsimd.partition_broadcast(rb[:], recip[:])
                    if half == 0:
                        nc.vector.tensor_tensor(
                            hid[0:64, hslot, :],
                            av[0:64, :], rb[:], op=mybir.AluOpType.mult)
                    else:
                        stg = norm_p.tile([64, SQ], f8, tag="stg")
                        nc.vector.tensor_tensor(
                            stg[:], av[0:64, :], rb[:],
                            op=mybir.AluOpType.mult)
                        nc.scalar.dma_start(hid[64:128, hslot, :], stg[:])

            # ---- output projection (fp8 DoubleRow over head-pair planes);
            # the 1/HS hidden scale and bo' ride the output activation.
            for ot in range(8):
                pso = sc_ps.tile([128, SQ], f32, tag="sc")
                for tp in range(4):
                    mmdr(pso[:],
                         wo_s[:, 2 * tp:2 * tp + 2, ot * 128:(ot + 1) * 128],
                         hiddens[tp][:, :, :],
                         start=(tp == 0), stop=(tp == 3))
                o_s = outs_p.tile([128, SQ], bf16, tag="outs")
                nc.scalar.activation(o_s[:], pso[:], Ident, scale=1.0 / HS,
                                     bias=bo_s[:, ot:ot + 1])
                nc.scalar.dma_start(outT[ot * 128:(ot + 1) * 128, :], o_s[:])

    nc.compile()
    return nc


def _get_nc():
    if "nc" not in _cache:
        _cache["nc"] = _build_program()
    return _cache["nc"]


def _prep_consts(Wq, bq, Wk, bk, Wv, bv, Wo, bo):
    import ml_dtypes
    f = np.float32
    b16 = ml_dtypes.bfloat16
    f8 = ml_dtypes.float8_e4m3

    def blockdiag2(W):
        out = np.zeros((128, 128), f)
        out[:64, :64] = W
        out[64:, 64:] = W
        return out

    Wqk = Wq.T @ Wk            # [d_in, m]: qh' = q Wqk + bq Wk
    bqk = bq @ Wk
    bv_full = np.tile(bv.astype(f), H)
    bo_adj = bo.astype(f) + Wo.astype(f) @ bv_full
    return {
        "wq2": blockdiag2(Wqk).astype(b16),
        "bq2": np.tile(bqk.astype(f), 2)[:, None].copy(),
        "wv2": blockdiag2(Wv.T).astype(b16),
        "wo8": np.ascontiguousarray(Wo.T.astype(f)).astype(f8),
        "bod": bo_adj[:, None].copy(),
    }


def kernel(q, k, v, Wq, bq, Wk, bk, Wv, bv, Wo, bo, _trace=False):
    import ml_dtypes
    b16 = ml_dtypes.bfloat16
    q = np.asarray(q, np.float32)
    k = np.asarray(k, np.float32)
    v = np.asarray(v, np.float32)
    consts = _prep_consts(
        np.asarray(Wq, np.float32), np.asarray(bq, np.float32),
        np.asarray(Wk, np.float32), np.asarray(bk, np.float32),
        np.asarray(Wv, np.float32), np.asarray(bv, np.float32),
        np.asarray(Wo, np.float32), np.asarray(bo, np.float32))

    in_maps = []
    for c in range(N_CORES):
        b, chunk = c // 2, c % 2
        m = dict(consts)
        m["qT"] = np.ascontiguousarray(
            q[b, chunk * SQ:(chunk + 1) * SQ, :].T).astype(b16)
        m["kT"] = np.ascontiguousarray(k[b].T).astype(b16)
        m["vT"] = np.ascontiguousarray(v[b].T).astype(b16)
        in_maps.append(m)

    nc = _get_nc()
    from concourse.bass_utils import run_bass_kernel_spmd
    res = run_bass_kernel_spmd(nc, in_maps, core_ids=list(range(N_CORES)),
                               trace=_trace)
    if _trace:
        kernel.last_results = res

    out = np.empty((B, S, D), np.float32)
    for c in range(N_CORES):
        b, chunk = c // 2, c % 2
        out[b, chunk * SQ:(chunk + 1) * SQ, :] = \
            res.results[c]["outT"].T.astype(np.float32)
    return out
